# revision 1
# baseline (speedup 1.0000x reference)
"""CWT latent attention kernel for 8 Trainium2 NeuronCores.

Sharding: core c = 2*b + g handles batch b and head-group g (8 of 16 heads).
Each core computes its heads' q/k/v, causal attention, and a partial output
projection (contracted over its heads' channels); the host sums the two
partials per batch.

Device layout notes:
- All matmul operands live "transposed" (contraction dim on partitions);
  activations are fed pre-transposed from the host, so no on-device
  transposes are needed anywhere.
- Attention computes scoresT = K^T-tile.T @ Q (shape [s, tq]); softmax skips
  the max-subtraction (scores are O(10), exp cannot overflow in fp32), the
  causal mask is a 0/1 multiply on diagonal tiles, and the denominator is a
  ones-vector matmul accumulated alongside the A@V matmul. The division is
  applied to the attention output tiles via a partition-broadcast of 1/den.
- RoPE is applied in [dh, t] layout as rot = x*C + swap(x*S'), where C/S'
  are host-precomputed [128, T] tables and swap exchanges adjacent partition
  pairs via a PE matmul with a pair-swap permutation matrix.
- Matmuls run in float32r (TF32); inputs are TF32-rounded on the host, and
  every on-device tensor feeding a matmul is written with dtype float32r.
"""

import math
from dataclasses import dataclass

import numpy as np

import concourse.bass as bass
import concourse.mybir as mybir
import concourse.tile as tile
from concourse import bacc
from concourse.bass_utils import run_bass_kernel_spmd

F32 = mybir.dt.float32
F32R = mybir.dt.float32r
EXP = mybir.ActivationFunctionType.Exp
SQRT = mybir.ActivationFunctionType.Sqrt
MUL = mybir.AluOpType.mult
ADD = mybir.AluOpType.add

# problem constants
B, T, D = 4, 2048, 2048
H, DH = 16, 128
D_LAT, D_HUB = 512, 1024
EPS = 1e-6
G = 2               # head groups == cores per batch
HG = H // G         # heads per core
NCORES = 8
TQ = 512            # tq group width for attention


@dataclass
class Cfg:
    T: int = T
    D: int = D
    DHUB: int = D_HUB
    DLAT: int = D_LAT
    HG: int = HG
    TH: int = 2      # t-stripes for XT residency in the q-projection

    @property
    def DC(self):  return self.D // 128        # xt chunks
    @property
    def HC(self):  return self.DHUB // 128     # hub chunks
    @property
    def LC(self):  return self.DLAT // 128     # latent chunks
    @property
    def ST(self):  return self.T // 128        # s tiles
    @property
    def NG(self):  return self.T // TQ         # tq groups
    @property
    def T4(self):  return self.T // 512        # 512-wide column chunks
    @property
    def THW(self): return self.T // self.TH    # t-half width
    @property
    def GD(self):  return self.HG * DH         # group channel width
    @property
    def EC(self):  return self.D // 512        # output e columns


def round_tf32(x: np.ndarray) -> np.ndarray:
    x = np.ascontiguousarray(x, np.float32)
    u = x.view(np.uint32)
    r = (u + 0x1000 + ((u >> 13) & 1)) & np.uint32(0xFFFFE000)
    return r.view(np.float32)


def input_specs(P: Cfg):
    return {
        "xt":      ([P.D, P.T], F32),
        "hubt":    ([P.DHUB, P.T], F32),
        "wqt":     ([P.D, P.GD], F32),
        "wdt":     ([P.DHUB, P.DLAT], F32),
        "wut_k":   ([P.DLAT, P.GD], F32),
        "wut_v":   ([P.DLAT, P.GD], F32),
        "wot":     ([P.GD, P.D], F32),
        "rope_c":  ([128, P.T], F32),
        "rope_sp": ([128, P.T], F32),
        "masks":   ([4, 128, TQ], F32),
        "ones_w":  ([128, 1], F32),
        "perm":    ([128, 128], F32),
    }


def _bcast_ap(row: bass.AP, p: int = 128) -> bass.AP:
    """[1, N] DRAM row -> partition-broadcast [p, N] read AP."""
    return bass.AP(tensor=row.tensor, offset=row.offset,
                   ap=[[0, p]] + [list(d) for d in row.ap[1:]])


def build_kernel(tc: tile.TileContext, outs: dict, ins: dict, P: Cfg, phases: str = "ABCDE"):
    nc = tc.nc
    scale = 1.0 / math.sqrt(DH)
    out_p = outs["out_p"]

    with tc.tile_pool(name="tables", bufs=1) as tables, \
         tc.tile_pool(name="dram", bufs=1, space="DRAM") as dram, \
         tc.tile_pool(name="ckvpool", bufs=1) as ckvpool:

        rope_c = tables.tile([128, P.T], F32)
        nc.sync.dma_start(out=rope_c, in_=ins["rope_c"][:])
        rope_sp = tables.tile([128, P.T], F32)
        nc.sync.dma_start(out=rope_sp, in_=ins["rope_sp"][:])
        # masks dram is [4,128,TQ]; load each mask as a [128, TQ] tile
        masks_t = [tables.tile([128, TQ], mybir.dt.bfloat16, name=f"mask{r}",
                               tag=f"mask{r}") for r in range(4)]
        for r in range(4):
            nc.gpsimd.dma_start(out=masks_t[r], in_=ins["masks"][r])
        ones_sb = tables.tile([128, 1], F32R)
        nc.sync.dma_start(out=ones_sb, in_=ins["ones_w"][:].bitcast(F32R))
        eps_sb = tables.tile([1, 1], F32)
        nc.vector.memset(eps_sb, EPS)
        perm_sb = tables.tile([128, 128], F32R)
        nc.sync.dma_start(out=perm_sb, in_=ins["perm"][:].bitcast(F32R))

        ckv = ckvpool.tile([128, P.LC, P.T], F32R)

        qspill = dram.tile([P.HG, 128, P.T], F32R)
        ospill = dram.tile([P.HG, 128, P.T], F32R)
        rms_dram = dram.tile([1, P.T], F32)

        # xt stripe 0, low-chunk half: prefetched before phase A so the
        # q-projection's first inputs land while A computes. Each stripe is
        # two 8-chunk tiles (xa = chunks 0..7, xb = 8..15) so only 32KB/part
        # coexists with phase A's pools.
        pbx = tc.alloc_tile_pool(name="pbx", bufs=1)
        DCH = P.DC // 2
        xt_tiles = {}

        def load_xt(th, half, pool=None):
            t0 = th * P.THW
            tag = "xa" if half == 0 else "xb"
            t_ = (pool or pbx).tile([128, DCH, P.THW], F32R, tag=tag,
                                    name=f"xt_{tag}{th}")
            for i in range(DCH):
                dc = half * DCH + i
                nc.sync.dma_start(
                    out=t_[:, i, :],
                    in_=ins["xt"][dc * 128:(dc + 1) * 128,
                                  t0:t0 + P.THW].bitcast(F32R))
            xt_tiles[(th, half)] = t_

        if "B" in phases:
            load_xt(0, 0)

        # ---------------- phase A: rms + c_kv ----------------
        if "A" not in phases:
            return
        with tc.tile_pool(name="pa", bufs=1) as pa, \
             tc.tile_pool(name="pa2", bufs=2) as pa2, \
             tc.tile_pool(name="psA", bufs=1, space="PSUM") as psA:
            hub_sb = pa.tile([128, P.HC, P.T], F32R)
            wdt_sb = pa.tile([128, P.HC, P.DLAT], F32R)
            for hc in range(P.HC):
                nc.sync.dma_start(out=hub_sb[:, hc, :],
                                  in_=ins["hubt"][hc * 128:(hc + 1) * 128, :].bitcast(F32R))
                nc.sync.dma_start(out=wdt_sb[:, hc, :],
                                  in_=ins["wdt"][hc * 128:(hc + 1) * 128, :].bitcast(F32R))

            ssq = [psA.tile([1, 512], F32, name=f"ssq{ts}", tag=f"ssq{ts}")
                   for ts in range(P.T4)]
            for hc in range(P.HC):
                for ts in range(P.T4):
                    sq = pa2.tile([128, 512], F32R, tag="sq")
                    nc.vector.tensor_tensor(sq[:], hub_sb[:, hc, ts * 512:(ts + 1) * 512],
                                            hub_sb[:, hc, ts * 512:(ts + 1) * 512], MUL)
                    nc.tensor.matmul(ssq[ts][:], ones_sb[:], sq[:],
                                     start=(hc == 0), stop=(hc == P.HC - 1))
            for ts in range(P.T4):
                srow = pa2.tile([1, 512], F32, tag="srow")
                nc.scalar.activation(srow[:], ssq[ts][:],
                                     SQRT, bias=eps_sb[:], scale=1.0 / P.DHUB)
                nc.vector.reciprocal_approx_fast(out=srow[:], in_=srow[:])
                nc.sync.dma_start(out=rms_dram[:, ts * 512:(ts + 1) * 512], in_=srow[:])
            rms_b = pa.tile([128, P.T], F32)
            nc.sync.dma_start(out=rms_b[:], in_=_bcast_ap(rms_dram[:]))

            for lc in range(P.LC):
                for tcol in range(P.T4):
                    ckvp = psA.tile([128, 512], F32, tag="ckvp", bufs=4)
                    for hc in range(P.HC):
                        nc.tensor.matmul(
                            ckvp[:],
                            wdt_sb[:, hc, lc * 128:(lc + 1) * 128],
                            hub_sb[:, hc, tcol * 512:(tcol + 1) * 512],
                            start=(hc == 0), stop=(hc == P.HC - 1))
                    nc.vector.tensor_tensor(
                        ckv[:, lc, tcol * 512:(tcol + 1) * 512],
                        ckvp[:], rms_b[:, tcol * 512:(tcol + 1) * 512], MUL)

        # ---------------- phase B: q projection + rope + spill ----------------
        if "B" not in phases:
            pbx.release()
            return
        with tc.tile_pool(name="pbB", bufs=1) as pbB, \
             tc.tile_pool(name="pb2", bufs=2) as pb2, \
             tc.tile_pool(name="psB", bufs=2, space="PSUM") as psB:
            for th in range(P.TH):
                t0 = th * P.THW
                for half in range(2):
                    if (th, half) not in xt_tiles:
                        load_xt(th, half, pool=(pbB if half else pbx))
                for h in range(P.HG):
                    wq_h = pb2.tile([128, P.DC, DH], F32R, tag="wq_h", bufs=4)
                    for dc in range(P.DC):
                        nc.sync.dma_start(
                            out=wq_h[:, dc, :],
                            in_=ins["wqt"][dc * 128:(dc + 1) * 128,
                                           h * DH:(h + 1) * DH].bitcast(F32R))
                    for t4 in range(P.THW // 512):
                        tq0 = t0 + t4 * 512
                        qp = psB.tile([128, 512], F32, tag="qp", bufs=4)
                        for dc in range(P.DC):
                            xt_t = xt_tiles[(th, dc // DCH)]
                            nc.tensor.matmul(qp[:], wq_h[:, dc, :],
                                             xt_t[:, dc % DCH, t4 * 512:(t4 + 1) * 512],
                                             start=(dc == 0), stop=(dc == P.DC - 1))
                        qc = pb2.tile([128, 512], F32, tag="qc")
                        nc.vector.tensor_tensor(qc[:], qp[:],
                                                rope_c[:, tq0:tq0 + 512], MUL)
                        qs = pb2.tile([128, 512], F32R, tag="qs")
                        nc.vector.tensor_tensor(qs[:], qp[:],
                                                rope_sp[:, tq0:tq0 + 512], MUL)
                        qw = psB.tile([128, 512], F32, tag="qw")
                        nc.tensor.matmul(qw[:], perm_sb[:], qs[:],
                                         start=True, stop=True)
                        qr = pb2.tile([128, 512], F32R, tag="qr", bufs=4)
                        nc.vector.tensor_tensor(qr[:], qc[:], qw[:], ADD)
                        nc.sync.dma_start(out=qspill[h, :, tq0:tq0 + 512], in_=qr[:])
        pbx.release()

        # ---------------- phase C/D: v, per-head k + attention ----------------
        if "C" not in phases:
            return
        with tc.tile_pool(name="pc", bufs=1) as pc, \
             tc.tile_pool(name="pd2", bufs=2) as pd2, \
             tc.tile_pool(name="pd3", bufs=3) as pd3:
            v_all = pc.tile([128, P.ST, P.GD], F32R)
            ncol = (P.GD + 511) // 512
            cw = min(512, P.GD)
            with tc.tile_pool(name="pcv", bufs=1) as pcv, \
                 tc.tile_pool(name="psC", bufs=2, space="PSUM") as psC:
                wut_v_sb = pcv.tile([128, P.LC, P.GD], F32R)
                for lc in range(P.LC):
                    nc.sync.dma_start(out=wut_v_sb[:, lc, :],
                                      in_=ins["wut_v"][lc * 128:(lc + 1) * 128, :].bitcast(F32R))
                for st in range(P.ST):
                    vps = psC.tile([128, P.GD], F32, tag="vps", bufs=3)
                    for lc in range(P.LC):
                        for hq in range(ncol):
                            nc.tensor.matmul(
                                vps[:, hq * cw:(hq + 1) * cw],
                                ckv[:, lc, st * 128:(st + 1) * 128],
                                wut_v_sb[:, lc, hq * cw:(hq + 1) * cw],
                                start=(lc == 0), stop=(lc == P.LC - 1))
                    nc.scalar.copy(out=v_all[:, st, :], in_=vps[:])


            with tc.tile_pool(name="psD", bufs=2, space="PSUM") as psD:
                for h in (range(P.HG) if "D" in phases else []):
                    # k projection + rope (per-head slice of W_up^T k-columns)
                    wk_h = pd2.tile([128, P.LC, DH], F32R, tag="wk_h")
                    for lc in range(P.LC):
                        nc.sync.dma_start(
                            out=wk_h[:, lc, :],
                            in_=ins["wut_k"][lc * 128:(lc + 1) * 128,
                                             h * DH:(h + 1) * DH].bitcast(F32R))
                    kT = pd2.tile([128, P.T], F32R, tag="kT", bufs=2)
                    for s4 in range(P.T4):
                        ks0 = s4 * 512
                        kps = psD.tile([128, 512], F32, tag="kps", bufs=1)
                        for lc in range(P.LC):
                            nc.tensor.matmul(kps[:],
                                             wk_h[:, lc, :],
                                             ckv[:, lc, ks0:ks0 + 512],
                                             start=(lc == 0), stop=(lc == P.LC - 1))
                        kc = pd2.tile([128, 512], F32, tag="kc")
                        nc.vector.tensor_tensor(kc[:], kps[:],
                                                rope_c[:, ks0:ks0 + 512], MUL)
                        ks = pd2.tile([128, 512], F32R, tag="ks")
                        nc.vector.tensor_tensor(ks[:], kps[:],
                                                rope_sp[:, ks0:ks0 + 512], MUL)
                        kw = psD.tile([128, 512], F32, tag="kps", bufs=1)
                        nc.tensor.matmul(kw[:], perm_sb[:], ks[:],
                                         start=True, stop=True)
                        nc.vector.tensor_tensor(kT[:, ks0:ks0 + 512], kc[:], kw[:], ADD)

                    qT = pd2.tile([128, P.T], F32R, tag="qT", bufs=2)
                    nc.sync.dma_start(out=qT[:], in_=qspill[h])

                    oT = pd2.tile([128, P.T], F32R, tag="oT", bufs=1)
                    for g in range(P.NG):
                        g0 = g * TQ
                        ops = psD.tile([128, TQ], F32, tag="ops")
                        denp = psD.tile([1, TQ], F32, tag="denp")
                        nchunk = (g + 1) * (TQ // 128)
                        for c in range(nchunk):
                            # causal: chunk c only contributes to tq >= c*128;
                            # narrow diagonal chunks to their valid tq suffix.
                            r = c - (g0 // 128)
                            off = max(r, 0) * 128
                            w = TQ - off
                            scp = psD.tile([128, TQ], F32, tag="scp", bufs=3)
                            nc.tensor.matmul(scp[:, :w],
                                             kT[:, c * 128:(c + 1) * 128],
                                             qT[:, g0 + off:g0 + TQ],
                                             start=True, stop=True)
                            pT = pd3.tile([128, TQ], F32R, tag="pT", bufs=4)
                            nc.scalar.activation(pT[:, :w], scp[:, :w], EXP, scale=scale)
                            if r >= 0:
                                # narrowed diagonal mask is always [tri|1...1]
                                nc.vector.tensor_tensor(pT[:, :w], pT[:, :w],
                                                        masks_t[0][:, :w], MUL)
                            nc.tensor.matmul(denp[:, off:], ones_sb[:], pT[:, :w],
                                             start=(c == 0), stop=(c == nchunk - 1))
                            nc.tensor.matmul(ops[:, off:],
                                             v_all[:, c, h * DH:(h + 1) * DH],
                                             pT[:, :w],
                                             start=(c == 0), stop=(c == nchunk - 1))
                        den_sb = pd2.tile([1, TQ], F32, tag="den_sb")
                        nc.scalar.copy(out=den_sb[:], in_=denp[:])
                        den_r = pd2.tile([1, TQ], F32, tag="den_r")
                        nc.vector.reciprocal_approx_fast(out=den_r[:], in_=den_sb[:])
                        den_dram = dram.tile([1, TQ], F32, tag="den_dram", bufs=4)
                        nc.sync.dma_start(out=den_dram[:], in_=den_r[:])
                        den_b = pd2.tile([128, TQ], F32, tag="den_b")
                        nc.sync.dma_start(out=den_b[:], in_=_bcast_ap(den_dram[:]))
                        nc.vector.tensor_tensor(oT[:, g0:g0 + TQ], ops[:], den_b[:], MUL)
                    nc.sync.dma_start(out=ospill[h], in_=oT[:])

        # ---------------- phase E: output projection ----------------
        if "E" not in phases:
            return
        with tc.tile_pool(name="pe", bufs=1) as pe, \
             tc.tile_pool(name="pe2", bufs=2) as pe2, \
             tc.tile_pool(name="psE", bufs=2, space="PSUM") as psE:
            wot_sb = pe.tile([128, P.HG, P.D], F32R)
            for h in range(P.HG):
                nc.sync.dma_start(out=wot_sb[:, h, :],
                                  in_=ins["wot"][h * 128:(h + 1) * 128, :].bitcast(F32R))
            for tt in range(P.ST):
                oth = pe2.tile([128, P.HG, DH], F32R, tag="oth", bufs=4)
                for h in range(P.HG):
                    nc.sync.dma_start(out=oth[:, h, :],
                                      in_=ospill[h, :, tt * 128:(tt + 1) * 128])
                out_sb = pe2.tile([128, P.D], F32, tag="out_sb", bufs=3)
                for ec in range(P.EC):
                    outp = psE.tile([128, 512], F32, tag="outp")
                    for h in range(P.HG):
                        nc.tensor.matmul(outp[:], oth[:, h, :],
                                         wot_sb[:, h, ec * 512:(ec + 1) * 512],
                                         start=(h == 0), stop=(h == P.HG - 1))
                    nc.scalar.copy(out=out_sb[:, ec * 512:(ec + 1) * 512], in_=outp[:])
                    nc.sync.dma_start(
                        out=out_p[tt * 128:(tt + 1) * 128, ec * 512:(ec + 1) * 512],
                        in_=out_sb[:, ec * 512:(ec + 1) * 512])


def make_rope_tables(cos: np.ndarray, sin: np.ndarray):
    """cos/sin [T, DH/2] -> C, S' [128, T] (fp32)."""
    Tn = cos.shape[0]
    C = np.repeat(np.ascontiguousarray(cos.T), 2, axis=0).astype(np.float32)
    SP = np.empty((DH, Tn), np.float32)
    SP[0::2] = sin.T
    SP[1::2] = -sin.T
    return C, SP


def make_masks():
    m = np.zeros((4, 128, TQ), np.float32)
    for r in range(4):
        for j in range(TQ):
            ti = j // 128
            if ti > r:
                m[r, :, j] = 1.0
            elif ti == r:
                m[r, :j % 128 + 1, j] = 1.0
    return m


def make_core_inputs(H_q, hub_content, Wq, norm_w, W_down, W_up, Wo, cos, sin):
    """Full inputs -> per-core in_maps (list of 8 dicts)."""
    C, SP = make_rope_tables(np.asarray(cos), np.asarray(sin))
    masks = make_masks()
    ones_w = np.ones((128, 1), np.float32)
    perm = np.zeros((128, 128), np.float32)
    for i in range(0, 128, 2):
        perm[i, i + 1] = 1.0
        perm[i + 1, i] = 1.0
    wdt = round_tf32((np.asarray(W_down) * np.asarray(norm_w)[None, :]).T)
    wut = np.asarray(W_up).T  # [D_LAT, 2D]
    in_maps = []
    for b in range(B):
        xt = round_tf32(np.asarray(H_q[b]).T)
        hubt = round_tf32(np.asarray(hub_content[b]).T)
        for g in range(G):
            gch = slice(g * HG * DH, (g + 1) * HG * DH)
            vch = slice(D + g * HG * DH, D + (g + 1) * HG * DH)
            in_maps.append({
                "xt": xt,
                "hubt": hubt,
                "wqt": round_tf32(np.asarray(Wq)[gch, :].T),
                "wdt": wdt,
                "wut_k": round_tf32(wut[:, gch]),
                "wut_v": round_tf32(wut[:, vch]),
                "wot": round_tf32(np.asarray(Wo)[:, gch].T),
                "rope_c": C,
                "rope_sp": SP,
                "masks": masks,
                "ones_w": ones_w,
                "perm": perm,
            })
    return in_maps


_NC_CACHE = {}


def _get_nc():
    if "nc" in _NC_CACHE:
        return _NC_CACHE["nc"]
    P = Cfg()
    nc = bacc.Bacc(None, target_bir_lowering=False)
    ins = {}
    for name, (shape, dt_) in input_specs(P).items():
        ins[name] = nc.dram_tensor(name, shape, dt_, kind="ExternalInput")[:]
    outs = {"out_p": nc.dram_tensor("out_p", [P.T, P.D], F32, kind="ExternalOutput")[:]}
    with tile.TileContext(nc) as tc:
        build_kernel(tc, outs, ins, P)
    nc.compile()
    _NC_CACHE["nc"] = nc
    return nc


def kernel(H_q, hub_content, Wq, norm_w, W_down, W_up, Wo, cos, sin):
    nc = _get_nc()
    in_maps = make_core_inputs(H_q, hub_content, Wq, norm_w, W_down, W_up,
                               Wo, cos, sin)
    res = run_bass_kernel_spmd(nc, in_maps, core_ids=list(range(NCORES)))
    out = np.empty((B, T, D), np.float32)
    for b in range(B):
        out[b] = res.results[2 * b]["out_p"] + res.results[2 * b + 1]["out_p"]
    return out



# revision 32
# speedup vs baseline: 6.1667x; 6.1667x over previous
"""CWT latent attention kernel for 8 Trainium2 NeuronCores.

Sharding: core c = 2*b + g handles batch b and head-group g (8 of 16 heads).
Each core computes its heads' q/k/v, causal attention, and a partial output
projection (contracted over its heads' channels); the host sums the two
partials per batch.

Design notes (v2):
- All matmul operands are bf16 (host-converted); PSUM accumulation is fp32.
  The correctness gate is rel_err < 2e-2; bf16 lands ~1e-2 of headroom away.
- Everything stays SBUF-resident: qT/kT/v_all for all 8 heads are bf16
  ([128, 8*2048] / [128, 16*1024] = 32 KB/partition each), so there are no
  DRAM spill round-trips between phases.
- DMAs are few and large (multi-dim access patterns: whole weight matrices
  in 1-2 transfers), issued on the otherwise-idle sync (loads) and gpsimd
  (stores) queues so no compute engine's sequencer stalls on DMA issue.
- Partition-broadcasts of per-column rows (1/rms, 1/denominator) are done
  with K=1 matmuls against a [1,128] ones row (fp32r, full PE rate at 512
  columns) instead of DRAM round-trips.
- softmax skips max-subtraction (scores are O(10), fp32 exp cannot
  overflow); causal mask is a 0/1 multiply on diagonal tiles; denominator
  accumulates via a ones-vector matmul alongside the A@V matmul.
- RoPE is applied in [dh, t] layout as rot = x*C + swap(x)*S2, where C/S2
  are host-precomputed [128, T] tables (S2 pre-swapped so the sign pattern
  lands right) and swap exchanges adjacent partition pairs via a DVE
  stream_shuffle (32-lane pair-swap mask), keeping the PE out of it.
- The output projection is interleaved with attention at tq-group (512)
  granularity: once all 8 heads finish a group, its 4 t-tiles are projected
  and stored, so the PE stream stays dense to the end.
"""

import math
from dataclasses import dataclass

import numpy as np
import ml_dtypes

import concourse.bass as bass
import concourse.mybir as mybir
import concourse.tile as tile
from concourse import bacc
from concourse.bass_utils import run_bass_kernel_spmd

F32 = mybir.dt.float32
F32R = mybir.dt.float32r
BF16 = mybir.dt.bfloat16
EXP = mybir.ActivationFunctionType.Exp
SQRT = mybir.ActivationFunctionType.Sqrt
MUL = mybir.AluOpType.mult
ADD = mybir.AluOpType.add
BFNP = ml_dtypes.bfloat16

# problem constants
B, T, D = 4, 2048, 2048
H, DH = 16, 128
D_LAT, D_HUB = 512, 1024
EPS = 1e-6
G = 2               # head groups == cores per batch
HG = H // G         # heads per core
NCORES = 8
TQ = 512            # tq group width for attention


@dataclass
class Cfg:
    T: int = T
    D: int = D
    DHUB: int = D_HUB
    DLAT: int = D_LAT
    HG: int = HG

    @property
    def DC(self):  return self.D // 128        # xt chunks
    @property
    def HC(self):  return self.DHUB // 128     # hub chunks
    @property
    def LC(self):  return self.DLAT // 128     # latent chunks
    @property
    def ST(self):  return self.T // 128        # s tiles
    @property
    def NG(self):  return self.T // TQ         # tq groups
    @property
    def T4(self):  return self.T // 512        # 512-wide column chunks
    @property
    def GD(self):  return self.HG * DH         # group channel width
    @property
    def EC(self):  return self.D // 512        # output e columns


def input_specs(P: Cfg):
    return {
        "xt":       ([P.D, P.T], BF16),
        "hubt":     ([P.DHUB, P.T], BF16),
        "wqt":      ([P.D, P.GD], BF16),
        "wdt":      ([P.DHUB, P.DLAT], BF16),
        "wut_k":    ([P.DLAT, P.GD], BF16),
        "wut_v":    ([P.DLAT, P.GD], BF16),
        "wot":      ([P.GD, P.D], BF16),
        "rope_c":   ([128, P.T], F32),
        "rope_sp":  ([128, P.T], F32),
        "mask0":    ([128, TQ], BF16),
        "ones_col": ([128, 1], BF16),
        "ones_row": ([1, 128], F32),
        "perm":     ([128, 128], BF16),
    }


def _ap3(src: bass.AP, row0: int, nch: int, col0: int, ncols: int, rowlen: int):
    """DRAM [R, rowlen] rows [row0, row0+128*nch) as a [128, nch, ncols] AP."""
    return bass.AP(tensor=src.tensor, offset=row0 * rowlen + col0,
                   ap=[[rowlen, 128], [128 * rowlen, nch], [1, ncols]])


def build_kernel(tc: tile.TileContext, outs: dict, ins: dict, P: Cfg, phases: str = "ABCDE"):
    nc = tc.nc
    scale = 1.0 / math.sqrt(DH)
    out_p = outs["out_p"]
    NSTR = P.T // 512          # xt stripes for the q projection

    with tc.tile_pool(name="tables", bufs=1) as tables:
        # ---- prefetch: small tables on the (otherwise store-only) gpsimd
        # queue, big loads on sync, ordered by first use ----
        ones_col = tables.tile([128, 1], BF16)
        nc.gpsimd.dma_start(out=ones_col, in_=ins["ones_col"][:])
        ones_row = tables.tile([1, 128], F32R)
        nc.gpsimd.dma_start(out=ones_row, in_=ins["ones_row"][:].bitcast(F32R))
        mask0 = tables.tile([128, TQ], BF16)
        nc.gpsimd.dma_start(out=mask0, in_=ins["mask0"][:])
        eps_sb = tables.tile([1, 1], F32)
        nc.vector.memset(eps_sb, EPS)

        # long-lived pools, bottom of the allocation stack. pshare's slots are
        # time-shared via tags: s1 = hub (phase A) then kT (phase C+),
        # s2 = wdt (phase A) then v_all (phase C+) — lifetimes are disjoint.
        pqt = tc.alloc_tile_pool(name="pqt", bufs=1)
        qT = pqt.tile([128, P.HG, P.T], BF16)
        pshare = tc.alloc_tile_pool(name="pshare", bufs=1)
        pckv = tc.alloc_tile_pool(name="pckv", bufs=1)
        ckv = pckv.tile([128, P.LC, P.T], BF16)
        pwq = tc.alloc_tile_pool(name="pwq", bufs=1)

        hub_sb = pshare.tile([128, P.HC, P.T], BF16, tag="s1", name="hub_sb")
        # chunks alternate between the sync and scalar DMA queues so the hub
        # streams in at 2x one queue's rate; first 512 columns of chunk 0
        # land first so phase A's first matmul starts ~2us sooner.
        nc.sync.dma_start(out=hub_sb[:, 0, 0:512],
                          in_=_ap3(ins["hubt"], 0, 1, 0, 512, P.T))
        nc.scalar.dma_start(out=hub_sb[:, 0, 512:],
                            in_=_ap3(ins["hubt"], 0, 1, 512, P.T - 512, P.T))
        for hc in range(1, P.HC):
            eng = nc.sync if hc % 2 == 0 else nc.scalar
            eng.dma_start(out=hub_sb[:, hc, :],
                          in_=_ap3(ins["hubt"], hc * 128, 1, 0, P.T, P.T))
        wdt_sb = pshare.tile([128, P.HC, P.DLAT], BF16, tag="s2", name="wdt_sb")
        nc.gpsimd.dma_start(out=wdt_sb,
                            in_=_ap3(ins["wdt"], 0, P.HC, 0, P.DLAT, P.DLAT))
        rope_c = tables.tile([128, P.T], F32)
        nc.gpsimd.dma_start(out=rope_c, in_=ins["rope_c"][:])
        rope_sp = tables.tile([128, P.T], F32)
        nc.gpsimd.dma_start(out=rope_sp, in_=ins["rope_sp"][:])

        wq_sb = pwq.tile([128, P.DC, P.GD], BF16)
        for half in range(2):
            nc.sync.dma_start(
                out=wq_sb[:, half * 8:(half + 1) * 8, :],
                in_=_ap3(ins["wqt"], half * 8 * 128, 8, 0, P.GD, P.GD))

        # ---------------- phase A: rms + c_kv ----------------
        if "A" in phases:
            with tc.tile_pool(name="pa2", bufs=3) as pa2, \
                 tc.tile_pool(name="parow", bufs=1) as parow, \
                 tc.tile_pool(name="psA", bufs=1, space="PSUM") as psA:
                ssq = [psA.tile([1, 512], F32, name=f"ssq{ts}", tag="ssq", bufs=4)
                       for ts in range(P.T4)]
                for hc in range(P.HC):
                    for ts in range(P.T4):
                        sq = pa2.tile([128, 512], BF16, tag="sq")
                        nc.vector.tensor_tensor(sq[:], hub_sb[:, hc, ts * 512:(ts + 1) * 512],
                                                hub_sb[:, hc, ts * 512:(ts + 1) * 512], MUL)
                        nc.tensor.matmul(ssq[ts][:], ones_col[:], sq[:],
                                         start=(hc == 0), stop=(hc == P.HC - 1))
                rms_b = parow.tile([128, P.T], F32)
                srows = []
                for ts in range(P.T4):
                    srow = pa2.tile([1, 512], F32, tag="srow", bufs=4)
                    nc.scalar.activation(srow[:], ssq[ts][:],
                                         SQRT, bias=eps_sb[:], scale=1.0 / P.DHUB)
                    nc.vector.reciprocal_approx_fast(out=srow[:], in_=srow[:])
                    # fp32r matmul operands must go through an explicit
                    # tf32-rounding op
                    srow_r = pa2.tile([1, 512], F32R, tag="srow_r", bufs=4)
                    nc.scalar.copy(out=srow_r[:], in_=srow[:])
                    srows.append(srow_r)

                # all 4 t-columns accumulate together so each wdt chunk is
                # loaded into the PE array once (4 matmuls per LDWEIGHTS).
                # The 1/rms broadcast matmuls are emitted after ckv's first
                # block so the PE isn't waiting on the reciprocal chain.
                for lc in range(P.LC):
                    ckvp = [psA.tile([128, 512], F32, tag="ckvp", bufs=4,
                                     name=f"ckvp{lc}_{t}") for t in range(P.T4)]
                    for hc in range(P.HC):
                        for tcol in range(P.T4):
                            nc.tensor.matmul(
                                ckvp[tcol][:],
                                wdt_sb[:, hc, lc * 128:(lc + 1) * 128],
                                hub_sb[:, hc, tcol * 512:(tcol + 1) * 512],
                                start=(hc == 0), stop=(hc == P.HC - 1))
                    if lc == 0:
                        for ts in range(P.T4):
                            # ssq slots are free; broadcast 1/rms via K=1 matmul
                            rbp = psA.tile([128, 512], F32, tag="ssq", bufs=4,
                                           name=f"rbp{ts}")
                            nc.tensor.matmul(rbp[:], ones_row[:], srows[ts][:],
                                             start=True, stop=True)
                            nc.scalar.copy(out=rms_b[:, ts * 512:(ts + 1) * 512],
                                           in_=rbp[:])
                    for tcol in range(P.T4):
                        nc.vector.tensor_tensor(
                            ckv[:, lc, tcol * 512:(tcol + 1) * 512],
                            ckvp[tcol][:], rms_b[:, tcol * 512:(tcol + 1) * 512], MUL)

        # ---------------- phase B: q projection + rope ----------------
        # 1024-wide t-stripes: each wq chunk LDWEIGHTS serves 2 matmuls.
        # RoPE for head h is deferred into head h+1's matmul stream so the
        # PE never waits on the DVE rope multiplies.
        if "B" in phases:
            with tc.tile_pool(name="pb2", bufs=2) as pb2, \
                 tc.tile_pool(name="psB", bufs=1, space="PSUM") as psB:
                rope_fin = None
                for ts in range(2):
                    t0 = ts * 1024
                    xt_s = pshare.tile([128, P.DC, 1024], BF16,
                                       tag=("s1" if ts == 0 else "s2"),
                                       name=f"xt{ts}")
                    for qtr in range(4):
                        nc.sync.dma_start(
                            out=xt_s[:, qtr * 4:(qtr + 1) * 4, :],
                            in_=_ap3(ins["xt"], qtr * 4 * 128, 4, t0, 1024, P.T))
                    for h in range(P.HG):
                        qp = [psB.tile([128, 512], F32, tag="qp", bufs=4,
                                       name=f"qp{ts}_{h}_{i}") for i in range(2)]
                        for dc in range(P.DC):
                            for i in range(2):
                                nc.tensor.matmul(
                                    qp[i][:], wq_sb[:, dc, h * DH:(h + 1) * DH],
                                    xt_s[:, dc, i * 512:(i + 1) * 512],
                                    start=(dc == 0), stop=(dc == P.DC - 1))
                        if rope_fin is not None:
                            rope_fin()

                        def rope_fin(qp=qp, h=h, t0=t0):
                            qcs, qss = [], []
                            for i in range(2):
                                tq0 = t0 + i * 512
                                qc = pb2.tile([128, 512], F32, tag="qc")
                                nc.vector.tensor_tensor(qc[:], qp[i][:],
                                                        rope_c[:, tq0:tq0 + 512], MUL)
                                qs = pb2.tile([128, 512], BF16, tag="qs")
                                nc.vector.tensor_tensor(qs[:], qp[i][:],
                                                        rope_sp[:, tq0:tq0 + 512], MUL)
                                qcs.append(qc)
                                qss.append(qs)
                            qws = [psB.tile([128, 512], F32, tag="qw", bufs=4,
                                            name=f"qw{ts}_{h}_{i}") for i in range(2)]
                            for i in range(2):
                                nc.tensor.matmul(qws[i][:], perm[:], qss[i][:],
                                                 start=True, stop=True)
                            for i in range(2):
                                tq0 = t0 + i * 512
                                nc.vector.tensor_tensor(qT[:, h, tq0:tq0 + 512],
                                                        qcs[i][:], qws[i][:], ADD)
                rope_fin()
        pwq.release()  # top of outer stack: tables, pqt, pshare, pckv, pwq

        # ---------------- phase C: v up-projection + k projection + rope ----
        v_all = pshare.tile([128, P.ST, P.GD], BF16, tag="s2", name="v_all")
        kT = pshare.tile([128, P.HG, P.T], BF16, tag="s1", name="kT")
        if "C" in phases:
            with tc.tile_pool(name="pcw", bufs=1) as pcw, \
                 tc.tile_pool(name="pc2", bufs=2) as pc2, \
                 tc.tile_pool(name="psC", bufs=1, space="PSUM") as psC:
                wut_v_sb = pcw.tile([128, P.LC, P.GD], BF16)
                nc.sync.dma_start(out=wut_v_sb,
                                  in_=_ap3(ins["wut_v"], 0, P.LC, 0, P.GD, P.GD))
                wut_k_sb = pcw.tile([128, P.LC, P.GD], BF16)
                nc.sync.dma_start(out=wut_k_sb,
                                  in_=_ap3(ins["wut_k"], 0, P.LC, 0, P.GD, P.GD))
                for st in range(P.ST):
                    vps = psC.tile([128, P.GD], F32, tag="vps", bufs=2)
                    for lc in range(P.LC):
                        for hq in range(2):
                            nc.tensor.matmul(
                                vps[:, hq * 512:(hq + 1) * 512],
                                ckv[:, lc, st * 128:(st + 1) * 128],
                                wut_v_sb[:, lc, hq * 512:(hq + 1) * 512],
                                start=(lc == 0), stop=(lc == P.LC - 1))
                    nc.scalar.copy(out=v_all[:, st, :], in_=vps[:])

                krope_fin = None
                for h in range(P.HG):
                    for s4 in range(P.T4):
                        kps = psC.tile([128, 512], F32, tag="kps", bufs=2)
                        for lc in range(P.LC):
                            nc.tensor.matmul(kps[:],
                                             wut_k_sb[:, lc, h * DH:(h + 1) * DH],
                                             ckv[:, lc, s4 * 512:(s4 + 1) * 512],
                                             start=(lc == 0), stop=(lc == P.LC - 1))
                        if krope_fin is not None:
                            krope_fin()

                        def krope_fin(kps=kps, h=h, s4=s4):
                            ks0 = s4 * 512
                            kc = pc2.tile([128, 512], F32, tag="kc")
                            nc.vector.tensor_tensor(kc[:], kps[:],
                                                    rope_c[:, ks0:ks0 + 512], MUL)
                            ks = pc2.tile([128, 512], BF16, tag="ks")
                            nc.vector.tensor_tensor(ks[:], kps[:],
                                                    rope_sp[:, ks0:ks0 + 512], MUL)
                            kw = psC.tile([128, 512], F32, tag="kw", bufs=2)
                            nc.tensor.matmul(kw[:], perm[:], ks[:],
                                             start=True, stop=True)
                            nc.vector.tensor_tensor(kT[:, h, ks0:ks0 + 512],
                                                    kc[:], kw[:], ADD)
                krope_fin()
        pckv.release()

        # ---------------- phase D/E: attention + output projection --------
        if "D" not in phases:
            return
        with tc.tile_pool(name="pwot", bufs=1) as pwot, \
             tc.tile_pool(name="pd2", bufs=2) as pd2, \
             tc.tile_pool(name="pd4", bufs=4) as pd4, \
             tc.tile_pool(name="pot", bufs=1) as pot, \
             tc.tile_pool(name="pout", bufs=2) as pout, \
             tc.tile_pool(name="psD", bufs=1, space="PSUM") as psD:
            wot_sb = pwot.tile([128, P.HG, P.D], BF16)
            for half in range(2):
                nc.sync.dma_start(
                    out=wot_sb[:, half * 4:(half + 1) * 4, :],
                    in_=_ap3(ins["wot"], half * 4 * 128, 4, 0, P.D, P.D))

            for g in range(P.NG):
                g0 = g * TQ
                nchunk = (g + 1) * (TQ // 128)
                oT = pot.tile([128, P.HG, TQ], BF16, tag="oT")
                # finalize(h) = 1/den broadcast + oT write for head h; deferred
                # into head h+1's chunk stream so the PE never waits on the
                # den reciprocal chain. The dbp PSUM bank does double duty:
                # first den accumulates in its partition-0 row, then (after the
                # copy-out) the K=1 broadcast matmul overwrites the full bank.
                fin = None
                for h in range(P.HG):
                    ops = psD.tile([128, TQ], F32, tag="psop", bufs=4,
                                   name=f"ops{g}_{h}")
                    dbp = psD.tile([128, TQ], F32, tag="dbp", bufs=1,
                                   name=f"dbp{g}_{h}")
                    # software pipeline: scores/exp run two chunks ahead of
                    # the A@V matmul, hiding exp (+ diagonal-mask) latency.
                    # The batched denominator pass (one ones-LDWEIGHTS for the
                    # whole head) fires right after the last scores matmul —
                    # before the A@V tail — so the reciprocal chain has several
                    # matmuls of slack before fin() needs it.
                    pTs = {}

                    def av(c, last):
                        pT, off, w = pTs[c]
                        nc.tensor.matmul(ops[:, off:],
                                         v_all[:, c, h * DH:(h + 1) * DH],
                                         pT[:, :w],
                                         start=(c == 0), stop=last)

                    for c in range(nchunk):
                        # causal: chunk c only contributes to tq >= c*128;
                        # narrow diagonal chunks to their valid tq suffix.
                        r = c - (g0 // 128)
                        off = max(r, 0) * 128
                        w = TQ - off
                        scp = psD.tile([128, TQ], F32, tag="scp", bufs=3)
                        nc.tensor.matmul(scp[:, :w],
                                         kT[:, h, c * 128:(c + 1) * 128],
                                         qT[:, h, g0 + off:g0 + TQ],
                                         start=True, stop=True)
                        pT = pd4.tile([128, TQ], BF16, tag="pT", bufs=17)
                        nc.scalar.activation(pT[:, :w], scp[:, :w], EXP, scale=scale)
                        if r >= 0:
                            # narrowed diagonal mask is always [tri|1...1]
                            nc.vector.tensor_tensor(pT[:, :w], pT[:, :w],
                                                    mask0[:, :w], MUL)
                        pTs[c] = (pT, off, w)
                        if c == (3 if nchunk <= 4 else 5) and fin is not None:
                            fin()
                            fin = None
                        if c >= 2:
                            av(c - 2, last=False)
                    for c in range(nchunk - 2):
                        pT, off, w = pTs[c]
                        nc.tensor.matmul(dbp[0:1, off:], ones_col[:], pT[:, :w],
                                         start=(c == 0), stop=False)
                    for c in range(nchunk - 2, nchunk):
                        av(c, last=(c == nchunk - 1))
                        pT, off, w = pTs[c]
                        nc.tensor.matmul(dbp[0:1, off:], ones_col[:], pT[:, :w],
                                         start=False, stop=(c == nchunk - 1))
                    # eager reciprocal straight from the PSUM row (so it does
                    # not queue behind the next head's exps); the PE-side
                    # broadcast and the oT write stay deferred in fin().
                    den_sb = pd2.tile([1, TQ], F32, tag="den_sb")
                    nc.vector.reciprocal_approx_fast(out=den_sb[:], in_=dbp[0:1, :])
                    den_r = pd2.tile([1, TQ], F32R, tag="den_r")
                    nc.scalar.copy(out=den_r[:], in_=den_sb[:])

                    def fin(ops=ops, dbp=dbp, den_r=den_r, h=h):
                        nc.tensor.matmul(dbp[:], ones_row[:], den_r[:],
                                         start=True, stop=True)
                        # DVE can't read two PSUM operands; stage the broadcast
                        # in SBUF via the scalar engine first.
                        den_bb = pd2.tile([128, TQ], F32, tag="den_bb")
                        nc.scalar.copy(out=den_bb[:], in_=dbp[:])
                        nc.vector.tensor_tensor(oT[:, h, :], ops[:], den_bb[:], MUL)
                fin()

                if "E" not in phases:
                    continue
                for tloc in range(TQ // 128):
                    tt = g * (TQ // 128) + tloc
                    out_sb = pout.tile([128, P.D], F32, tag="out_sb")
                    for ecp in range(2):
                        # ec pairs: each oT LDWEIGHTS serves 2 matmuls
                        outp = [psD.tile([128, 512], F32, tag="psop", bufs=4,
                                         name=f"outp{tt}_{ecp}_{i}")
                                for i in range(2)]
                        for h in range(P.HG):
                            for i in range(2):
                                ec = ecp * 2 + i
                                nc.tensor.matmul(
                                    outp[i][:],
                                    oT[:, h, tloc * 128:(tloc + 1) * 128],
                                    wot_sb[:, h, ec * 512:(ec + 1) * 512],
                                    start=(h == 0), stop=(h == P.HG - 1))
                        for i in range(2):
                            ec = ecp * 2 + i
                            nc.scalar.copy(out=out_sb[:, ec * 512:(ec + 1) * 512],
                                           in_=outp[i][:])
                    nc.gpsimd.dma_start(
                        out=out_p[tt * 128:(tt + 1) * 128, :],
                        in_=out_sb[:])
        pshare.release()
        pqt.release()


def make_rope_tables(cos: np.ndarray, sin: np.ndarray):
    """cos/sin [T, DH/2] -> C, S' [128, T] (fp32)."""
    Tn = cos.shape[0]
    C = np.repeat(np.ascontiguousarray(cos.T), 2, axis=0).astype(np.float32)
    SP = np.empty((DH, Tn), np.float32)
    SP[0::2] = sin.T
    SP[1::2] = -sin.T
    return C, SP


def make_mask0():
    m = np.zeros((128, TQ), np.float32)
    for j in range(TQ):
        if j // 128 > 0:
            m[:, j] = 1.0
        else:
            m[:j % 128 + 1, j] = 1.0
    return m.astype(BFNP)


def make_core_inputs(H_q, hub_content, Wq, norm_w, W_down, W_up, Wo, cos, sin):
    """Full inputs -> per-core in_maps (list of 8 dicts)."""
    C, SP = make_rope_tables(np.asarray(cos), np.asarray(sin))
    mask0 = make_mask0()
    ones_col = np.ones((128, 1), BFNP)
    ones_row = np.ones((1, 128), np.float32)
    perm = np.zeros((128, 128), np.float32)
    for i in range(0, 128, 2):
        perm[i, i + 1] = 1.0
        perm[i + 1, i] = 1.0
    perm = perm.astype(BFNP)
    bf = lambda x: np.ascontiguousarray(x).astype(BFNP)
    wdt = bf((np.asarray(W_down) * np.asarray(norm_w)[None, :]).T)
    wut = np.asarray(W_up).T  # [D_LAT, 2D]
    in_maps = []
    for b in range(B):
        xt = bf(np.asarray(H_q[b]).T)
        hubt = bf(np.asarray(hub_content[b]).T)
        for g in range(G):
            gch = slice(g * HG * DH, (g + 1) * HG * DH)
            vch = slice(D + g * HG * DH, D + (g + 1) * HG * DH)
            in_maps.append({
                "xt": xt,
                "hubt": hubt,
                "wqt": bf(np.asarray(Wq)[gch, :].T),
                "wdt": wdt,
                "wut_k": bf(wut[:, gch]),
                "wut_v": bf(wut[:, vch]),
                "wot": bf(np.asarray(Wo)[:, gch].T),
                "rope_c": C,
                "rope_sp": SP,
                "mask0": mask0,
                "ones_col": ones_col,
                "ones_row": ones_row,
                "perm": perm,
            })
    return in_maps


_NC_CACHE = {}


def _get_nc(iters: int = 1):
    """Compile the kernel; iters>1 emits the whole computation that many
    times back-to-back (same inputs/outputs), for on-device timing that
    cancels per-dispatch RPC overhead."""
    key = ("nc", iters)
    if key in _NC_CACHE:
        return _NC_CACHE[key]
    P = Cfg()
    nc = bacc.Bacc(None, target_bir_lowering=False)
    ins = {}
    for name, (shape, dt_) in input_specs(P).items():
        ins[name] = nc.dram_tensor(name, shape, dt_, kind="ExternalInput")[:]
    outs = {"out_p": nc.dram_tensor("out_p", [P.T, P.D], F32, kind="ExternalOutput")[:]}
    with tile.TileContext(nc) as tc:
        for _ in range(iters):
            build_kernel(tc, outs, ins, P)
    nc.compile()
    _NC_CACHE[key] = nc
    return nc


def kernel(H_q, hub_content, Wq, norm_w, W_down, W_up, Wo, cos, sin):
    nc = _get_nc()
    in_maps = make_core_inputs(H_q, hub_content, Wq, norm_w, W_down, W_up,
                               Wo, cos, sin)
    res = run_bass_kernel_spmd(nc, in_maps, core_ids=list(range(NCORES)))
    out = np.empty((B, T, D), np.float32)
    for b in range(B):
        out[b] = res.results[2 * b]["out_p"] + res.results[2 * b + 1]["out_p"]
    return out
